# revision 1
# baseline (speedup 1.0000x reference)
"""Tensor-parallel causal attention block for Trainium2 (8 NeuronCores).

Sharding: tensor-parallel across heads for QKV+attention (2 heads/core),
then an AllToAll (fp16 payload, 4MB/core) to switch to row-parallel for
the output projection — much cheaper than the AllReduce the module's
TPLinear layout implies (64MB/core).

Dataflow per core: x^T is pre-transposed on the host so the C-contraction
sits on the partition axis. Q,K are produced transposed [d,t] with RoPE
fused into the PSUM eviction (cos/sin tables host-precomputed); V in
natural [t,d] layout. Scores are computed transposed (S^T = K·Q) so the
PV matmul needs no on-chip transposes at all. Softmax skips the
max-subtraction (scores are O(5) here, exp is fp32-safe), masks causality
with a host-built 0/1 tile (one sliced view per diagonal offset), gets the
denominator from a ones-vector matmul accumulated alongside PV, and folds
normalization into the PSUM eviction via reciprocal + a rank-1 broadcast
matmul.

All matmul inputs are fp16 (inputs cast on host, intermediates written as
fp16 by the evicting engine); accumulation stays fp32 in PSUM. Score
tiles are paired two-per-PSUM-allocation so one exp covers 1024 columns,
halving the S->exp->PV cross-engine sync hops.
"""
import numpy as np

import concourse.bass as bass
import concourse.tile as tile
import concourse.mybir as mybir
from concourse.bass_utils import run_bass_kernel_spmd

N_CORES = 8
B, T, C = 4, 2048, 2048
H = 16                 # total heads
HPC = H // N_CORES     # heads per core = 2
D = C // H             # head dim = 128
P = 128                # partitions
TG = 512               # t-group (moving free dim)
NTG = T // TG          # 4 groups per batch
NCC = C // P           # 16 contraction chunks
NSLICE = B * T // N_CORES  # 1024 output rows per core

FP = mybir.dt.float32
FPR = mybir.dt.float32r
FP16 = mybir.dt.float16
EXP = mybir.ActivationFunctionType.Exp
SCALE = 1.0 / float(np.sqrt(D))

# ---------------------------------------------------------------------------
# Workaround: this container's walrus rejects >1 sync-wait per instruction.
# Hoist extras onto preceding same-engine NoOps (engine streams are in-order).
# ---------------------------------------------------------------------------
from concourse.vector_clock import ScopedClock


def _fixup_multiwaits(nc):
    moved = 0
    for fn in nc.m.functions:
        for bb in fn.blocks:
            insts = bb.instructions
            if not any(
                i.sync_info and i.sync_info.on_wait and len(i.sync_info.on_wait) > 1
                for i in insts
            ):
                continue
            new_insts = []
            for ins in insts:
                si = ins.sync_info
                if si is not None and si.on_wait and len(si.on_wait) > 1:
                    extra, keep = si.on_wait[:-1], si.on_wait[-1:]
                    for w in extra:
                        nop = mybir.InstNoOp(
                            name=nc.get_next_instruction_name(),
                            ins=[],
                            outs=[],
                            engine=ins.engine,
                        )
                        nop.sync_info = mybir.SyncInfo(on_wait=[w], on_update=[])
                        new_insts.append(nop)
                        moved += 1
                    si.on_wait = keep
                new_insts.append(ins)
            bb.instructions = new_insts
    return moved


def _patched_drain_and_barrier(self, tick_clock, wait_clock):
    nop = self.nc.sync.nop(nofuse=True)
    wait_clock.add_sem_waits(nop.ins, ScopedClock({None: tick_clock.global_clock}))
    w = nop.ins.sync_info.on_wait if nop.ins.sync_info else []
    while w and len(w) > 1:
        cond = w.pop()
        n2 = self.nc.sync.nop(nofuse=True)
        if n2.ins.sync_info is None:
            n2.ins.sync_info = mybir.SyncInfo(on_wait=[], on_update=[])
        n2.ins.sync_info.on_wait.append(cond)
    self.nc.sync.drain()
    self.nc.all_engine_barrier()
    assert self.sems is not None
    popped = self.nc._tile_sem_poison_stack.pop()
    assert popped is self._sem_poison
    self.nc.clear_and_free_semaphores(list(self.sems.allocated().values()))
    self.nc.all_engine_barrier()


tile.TileContext._drain_and_barrier = _patched_drain_and_barrier

# SBUF cap: tile_utils caps at 192KB/partition; cayman has 208 usable.
try:
    import concourse.tile_utils as _tile_utils

    if getattr(_tile_utils, "max_sbuf_usage", None) is not None:
        _tile_utils.max_sbuf_usage = 204 * 1024
except Exception:
    pass


# ---------------------------------------------------------------------------
# Device program
# ---------------------------------------------------------------------------
def build_program(reps: int = 1, mode: str = "full"):
    nc = bass.Bass()

    xT = nc.dram_tensor("xT", [B, C, T], FP16, kind="ExternalInput")
    wqT = nc.dram_tensor("wqT", [C, HPC * D], FP16, kind="ExternalInput")
    wkT = nc.dram_tensor("wkT", [C, HPC * D], FP16, kind="ExternalInput")
    wvT = nc.dram_tensor("wvT", [C, HPC * D], FP16, kind="ExternalInput")
    woT = nc.dram_tensor("woT", [C, C], FP16, kind="ExternalInput")
    cos_t = nc.dram_tensor("cos_t", [D // 2, T], FP, kind="ExternalInput")
    sin_t = nc.dram_tensor("sin_t", [D // 2, T], FP, kind="ExternalInput")
    maskc = nc.dram_tensor("maskc", [P, 896], FP16, kind="ExternalInput")
    ones_col = nc.dram_tensor("ones_col", [P, 1], FP16, kind="ExternalInput")
    ones_row = nc.dram_tensor("ones_row", [1, P], FP16, kind="ExternalInput")

    out_rows = nc.dram_tensor("out_rows", [NSLICE, C], FP, kind="ExternalOutput")

    with tile.TileContext(nc) as tc:
        # ---- persistent constants -------------------------------------
        with (
            tc.tile_pool(name="const", bufs=1) as const,
            tc.tile_pool(name="wpool", bufs=1) as wpool,
        ):
            mask_s = const.tile([P, 896], FP16)
            ones_c = const.tile([P, 1], FP16)
            ones_r = const.tile([1, P], FP16)
            nc.sync.dma_start(mask_s[:], maskc[:])
            nc.sync.dma_start(ones_c[:], ones_col[:])
            nc.sync.dma_start(ones_r[:], ones_row[:])

            wq_s, wk_s, wv_s = [], [], []
            for cc in range(NCC):
                tq = wpool.tile([P, HPC * D], FP16, tag=f"wq{cc}", name=f"wq{cc}")
                tk = wpool.tile([P, HPC * D], FP16, tag=f"wk{cc}", name=f"wk{cc}")
                tv = wpool.tile([P, HPC * D], FP16, tag=f"wv{cc}", name=f"wv{cc}")
                nc.sync.dma_start(tq[:], wqT[P * cc : P * (cc + 1), :])
                nc.sync.dma_start(tk[:], wkT[P * cc : P * (cc + 1), :])
                nc.sync.dma_start(tv[:], wvT[P * cc : P * (cc + 1), :])
                wq_s.append(tq)
                wk_s.append(tk)
                wv_s.append(tv)

            for rep in range(reps):
                _emit_body(
                    nc, tc, rep, xT, woT, out_rows,
                    cos_t, sin_t, mask_s, ones_c, ones_r, wq_s, wk_s, wv_s,
                    mode=mode,
                )

    moved = _fixup_multiwaits(nc)
    return nc, moved


def _emit_body(nc, tc, rep, xT, woT, out_rows,
               cos_t, sin_t, mask_s, ones_c, ones_r, wq_s, wk_s, wv_s,
               mode="full"):
    HALF = D // 2
    sink = nc.dram_tensor(f"sink_{rep}", [P, 160 * 1024], FP16) if mode.startswith("proj") else None
    # A2A staging, split by head so the first collective can fire while the
    # last batch's h=1 attention is still running, and phase 3 can start
    # accumulating h=0 chunks while the second collective is in flight.
    # ya_in_h rows = 128*shard + d, cols = n within shard.
    ya_in_h = [
        nc.dram_tensor(f"ya_in_{rep}_{h}", [N_CORES * D, NSLICE], FP16)
        for h in range(HPC)
    ]
    ya_out_h = [
        nc.dram_tensor(f"ya_out_{rep}_{h}", [N_CORES * D, NSLICE], FP16)
        for h in range(HPC)
    ]

    with (
        tc.tile_pool(name="xt", bufs=2) as xt_pool,
        tc.tile_pool(name="qkv", bufs=2) as qkv_pool,
        tc.tile_pool(name="ptile", bufs=6) as p_pool,
        tc.tile_pool(name="evict", bufs=3) as e_pool,
        tc.tile_pool(name="small", bufs=2) as s_pool,
        tc.tile_pool(name="ps", bufs=1, space="PSUM") as ps,
    ):
        for b in range(B):
            # ---- QKV projections for batch b --------------------------
            qk_tiles = {}
            for pj in ("q", "k"):
                for h in range(HPC):
                    qk_tiles[(pj, h)] = qkv_pool.tile(
                        [P, T], FP16, tag=f"{pj}T{h}", name=f"{pj}T{h}_{b}"
                    )
            v_tiles = [
                qkv_pool.tile([P, HPC * D], FP16, tag=f"v{tch}", name=f"v{tch}_{b}")
                for tch in range(T // P)
            ]

            TGP = 2 * TG  # 1024-wide projection groups
            for tg in range(T // TGP):
                cos_sl = s_pool.tile([D // 2, TGP], FP, tag="cosS", name=f"cos_{b}_{tg}")
                sin_sl = s_pool.tile([D // 2, TGP], FP, tag="sinS", name=f"sin_{b}_{tg}")
                nc.sync.dma_start(cos_sl[:], cos_t[:, TGP * tg : TGP * (tg + 1)])
                nc.sync.dma_start(sin_sl[:], sin_t[:, TGP * tg : TGP * (tg + 1)])
                xts = []
                for cc in range(NCC):
                    xt = xt_pool.tile([P, TGP], FP16, tag=f"xt{cc}", name=f"xt{cc}_{b}_{tg}")
                    nc.sync.dma_start(
                        xt[:], xT[b, P * cc : P * (cc + 1), TGP * tg : TGP * (tg + 1)]
                    )
                    xts.append(xt)

                # q, k: transposed orientation [d, t] with fused RoPE evict
                for pj, wt in (("q", wq_s), ("k", wk_s)):
                    for h in range(HPC):
                        pmm = ps.tile([P, TGP], FP, tag="big2", bufs=2, name=f"p{pj}{h}_{b}_{tg}")
                        for half in range(2):
                            for cc in range(NCC):
                                nc.tensor.matmul(
                                    pmm[:, TG * half : TG * (half + 1)],
                                    wt[cc][:, D * h : D * (h + 1)],
                                    xts[cc][:, TG * half : TG * (half + 1)],
                                    start=(cc == 0),
                                    stop=(cc == NCC - 1),
                                )
                        dst = qk_tiles[(pj, h)]
                        t1 = s_pool.tile([HALF, TGP], FP, tag="ropeA", name=f"t1_{b}_{tg}")
                        t2 = s_pool.tile([HALF, TGP], FP, tag="ropeB", name=f"t2_{b}_{tg}")
                        x1 = pmm[0:HALF, :]
                        x2 = pmm[HALF:P, :]
                        dcol = dst[:, TGP * tg : TGP * (tg + 1)]
                        nc.vector.tensor_mul(t1[:], x1, cos_sl[:])
                        nc.vector.tensor_mul(t2[:], x2, sin_sl[:])
                        nc.vector.tensor_sub(dcol[0:HALF, :], t1[:], t2[:])
                        nc.vector.tensor_mul(t1[:], x1, sin_sl[:])
                        nc.vector.tensor_mul(t2[:], x2, cos_sl[:])
                        nc.vector.tensor_add(dcol[HALF:P, :], t1[:], t2[:])

                # v: natural orientation [t, d] for both heads
                for t4 in range(TGP // P):
                    tch = (TGP * tg) // P + t4
                    pv = ps.tile([P, HPC * D], FP, tag="misc", bufs=2, name=f"pv_{b}_{tch}")
                    for cc in range(NCC):
                        nc.tensor.matmul(
                            pv[:],
                            xts[cc][:, P * t4 : P * (t4 + 1)],
                            wv_s[cc][:],
                            start=(cc == 0),
                            stop=(cc == NCC - 1),
                        )
                    nc.scalar.copy(v_tiles[tch][:], pv[:])

            # ---- attention for batch b --------------------------------
            if mode.startswith("proj"):
                for idx, ((pj, h), tl) in enumerate(qk_tiles.items()):
                    nc.sync.dma_start(
                        sink[:, (4 * b + idx) * T : (4 * b + idx) * T + T], tl[:]
                    )
                for tch, vt in enumerate(v_tiles):
                    nc.sync.dma_start(
                        sink[:, 33 * T + (16 * b + tch) * HPC * D : 33 * T + (16 * b + tch) * HPC * D + HPC * D],
                        vt[:],
                    )
                continue
            for h in range(HPC):
                qT = qk_tiles[("q", h)]
                kT = qk_tiles[("k", h)]
                for g in range(NTG):
                    n_i = 4 * g + 4  # causal: tk chunks 0 .. 4g+3 (even count)
                    po = ps.tile([P, TG], FP, tag="acc512", bufs=2, name=f"po_{b}_{h}_{g}")
                    pd = ps.tile([1, TG], FP, tag="misc", bufs=2, name=f"pd_{b}_{h}_{g}")
                    for pi in range(n_i // 2):
                        pss = ps.tile([P, 2 * TG], FP, tag="big2", bufs=2, name=f"ps_{b}_{h}_{g}_{pi}")
                        for half in range(2):
                            i = 2 * pi + half
                            nc.tensor.matmul(
                                pss[:, TG * half : TG * (half + 1)],
                                kT[:, P * i : P * (i + 1)],
                                qT[:, TG * g : TG * (g + 1)],
                                start=True,
                                stop=True,
                            )
                        pt = p_pool.tile([P, 2 * TG], FP16, tag="pT", name=f"pt_{b}_{h}_{g}_{pi}")
                        nc.scalar.activation(pt[:], pss[:], EXP, scale=SCALE)
                        offs = []
                        for half in range(2):
                            i = 2 * pi + half
                            pth = pt[:, TG * half : TG * (half + 1)]
                            r = 0
                            if i >= 4 * g and "nomask" not in mode:
                                r = P * i - TG * g
                                nc.vector.tensor_mul(
                                    pth, pth, mask_s[:, 384 - r : 896 - r]
                                )
                            offs.append((i, pth, r))
                        # masked cols < r are zero: skip them. PVs batched
                        # before denoms so the ones stationary loads once/pair.
                        for i, pth, r in offs:
                            nc.tensor.matmul(
                                po[:, r:TG],
                                v_tiles[i][:, D * h : D * (h + 1)],
                                pth[:, r:TG],
                                start=(i == 0),
                                stop=(i == n_i - 1),
                            )
                        for i, pth, r in offs:
                            nc.tensor.matmul(
                                pd[:, r:TG],
                                ones_c[:],
                                pth[:, r:TG],
                                start=(i == 0),
                                stop=(i == n_i - 1),
                            )
                    recip = s_pool.tile([1, TG], FP16, tag="recip", name=f"rc_{b}_{h}_{g}")
                    with nc.allow_low_precision(reason="softmax denom recip; values O(1e3)"):
                        nc.vector.reciprocal(recip[:], pd[:])
                    prb = ps.tile([P, TG], FP, tag="acc512", bufs=2, name=f"prb_{b}_{h}_{g}")
                    nc.tensor.matmul(prb[:], ones_r[:], recip[:], start=True, stop=True)
                    rb = e_pool.tile([P, TG], FP, tag="rb", name=f"rb_{b}_{h}_{g}")
                    nc.scalar.copy(rb[:], prb[:])
                    yt = e_pool.tile([P, TG], FP16, tag="yt", name=f"yt_{b}_{h}_{g}")
                    nc.vector.tensor_mul(yt[:], po[:], rb[:])
                    # stage for A2A: shard j = n // NSLICE, col = n % NSLICE
                    n0 = T * b + TG * g
                    j = n0 // NSLICE
                    col = n0 % NSLICE
                    row = D * j
                    nc.sync.dma_start(
                        ya_in_h[h][row : row + D, col : col + TG], yt[:]
                    )

    # ---- AllToAll: head-sharded -> row-sharded ------------------------
    if mode.startswith(("proj", "attn")):
        return
    for h in range(HPC):
        nc.gpsimd.collective_compute(
            "AllToAll",
            mybir.AluOpType.bypass,
            replica_groups=[list(range(N_CORES))],
            ins=[ya_in_h[h][:]],
            outs=[ya_out_h[h][:]],
        )

    # ---- output projection on this core's row slice -------------------
    with (
        tc.tile_pool(name="ytp", bufs=1) as yt_pool,
        tc.tile_pool(name="wop", bufs=2) as wo_pool,
        tc.tile_pool(name="outp", bufs=4) as out_pool,
        tc.tile_pool(name="ps3", bufs=4, space="PSUM") as ps3,
    ):
        yts = {}
        for h in range(HPC):
            for j in range(N_CORES):
                cc = HPC * j + h
                yt = yt_pool.tile([P, NSLICE], FP16, tag=f"y{cc}", name=f"y{cc}_{rep}")
                nc.sync.dma_start(yt[:], ya_out_h[h][P * j : P * (j + 1), :])
                yts[cc] = yt
        TG3 = TG
        for jg in range(C // TG3):
            wos = []
            for cc in range(NCC):
                wo = wo_pool.tile([P, TG3], FP16, tag=f"wo{cc}", name=f"wo{cc}_{rep}_{jg}")
                nc.sync.dma_start(
                    wo[:], woT[P * cc : P * (cc + 1), TG3 * jg : TG3 * (jg + 1)]
                )
                wos.append(wo)
            cc_order = [HPC * j + h for h in range(HPC) for j in range(N_CORES)]
            for nt in range(NSLICE // P):
                pout = ps3.tile([P, TG3], FP, tag="out", bufs=4, name=f"pout_{rep}_{jg}_{nt}")
                for ci, cc in enumerate(cc_order):
                    nc.tensor.matmul(
                        pout[:],
                        yts[cc][:, P * nt : P * (nt + 1)],
                        wos[cc][:],
                        start=(ci == 0),
                        stop=(ci == NCC - 1),
                    )
                ot = out_pool.tile([P, TG3], FP, tag="ot", name=f"ot_{rep}_{jg}_{nt}")
                nc.scalar.copy(ot[:], pout[:])
                nc.sync.dma_start(
                    out_rows[P * nt : P * (nt + 1), TG3 * jg : TG3 * (jg + 1)], ot[:]
                )


# ---------------------------------------------------------------------------
# Host-side prep + execution
# ---------------------------------------------------------------------------
def _host_inputs(x, wq, wk, wv, wo):
    xT = np.ascontiguousarray(x.transpose(0, 2, 1)).astype(np.float16)
    woT = np.ascontiguousarray(wo.T).astype(np.float16)

    half = D // 2
    freqs = 1.0 / (10000.0 ** (np.arange(half, dtype=np.float32) / half))
    t = np.arange(T, dtype=np.float32)
    ang = freqs[:, None] * t[None, :]  # [half, T]
    cos_t = np.cos(ang).astype(np.float32)
    sin_t = np.sin(ang).astype(np.float32)

    # maskc[p, n] = 1.0 iff p <= n - 384  (sliced per diagonal offset)
    pp = np.arange(P)[:, None]
    nn = np.arange(896)[None, :]
    maskc = (pp <= nn - 384).astype(np.float16)

    ones_col = np.ones((P, 1), dtype=np.float16)
    ones_row = np.ones((1, P), dtype=np.float16)

    common = dict(
        xT=xT, woT=woT, cos_t=cos_t, sin_t=sin_t, maskc=maskc,
        ones_col=ones_col, ones_row=ones_row,
    )
    in_maps = []
    for r in range(N_CORES):
        rows = slice(HPC * D * r, HPC * D * (r + 1))
        in_maps.append(
            dict(
                common,
                wqT=np.ascontiguousarray(wq[rows, :].T).astype(np.float16),
                wkT=np.ascontiguousarray(wk[rows, :].T).astype(np.float16),
                wvT=np.ascontiguousarray(wv[rows, :].T).astype(np.float16),
            )
        )
    return in_maps


_CACHED = {}


def _get_program(reps=1):
    if reps not in _CACHED:
        _CACHED[reps] = build_program(reps)[0]
    return _CACHED[reps]


def kernel(x, wq, wk, wv, wo):
    nc = _get_program(1)
    in_maps = _host_inputs(
        np.asarray(x, dtype=np.float32),
        np.asarray(wq, dtype=np.float32),
        np.asarray(wk, dtype=np.float32),
        np.asarray(wv, dtype=np.float32),
        np.asarray(wo, dtype=np.float32),
    )
    res = run_bass_kernel_spmd(nc, in_maps, list(range(N_CORES)))
    out = np.concatenate([res.results[r]["out_rows"] for r in range(N_CORES)], axis=0)
    return out.reshape(B, T, C)



# revision 12
# speedup vs baseline: 1.4675x; 1.4675x over previous
"""Tensor-parallel causal attention block for Trainium2 (8 NeuronCores).

Sharding: tensor-parallel across heads (2 heads/core) for QKV+attention,
then one AllToAll per local head (fp16, 2MB) to switch to row-parallel
for the output projection.

Key structure (v2):
- Head-major attention passes: all batches' h=0 attention runs first, so
  AllToAll(h0) overlaps the entire h=1 pass; phase 3 leads with h0
  accumulation (8-pout stagger) to cover AllToAll(h1).
- PV matmul uses exp(S^T) sub-blocks as the STATIONARY operand and
  [V | ones] as the moving operand: the softmax denominator comes out of
  the same matmul pass (column 128), eliminating the separate
  ones-row denominator matmuls and the reciprocal-broadcast matmul.
  Output is naturally [q, d]; normalization is a per-partition
  tensor_scalar multiply; staging is written [n, d]-major and
  re-transposed after the collective with the DMA transpose XBAR.
- RoPE fused into the QKV PSUM eviction as 4 full-partition DVE ops
  using host-built [cos;cos] / [-sin;sin] tables (fp16, loaded once).
- Bulk single-instruction DMAs (weights, x tiles, wo, yt) to cut
  descriptor-generation cost; DMA emission ordered so the first matmul
  starts after ~1.5MB of transfers.
- Attention is software-pipelined: scores(k+1) is emitted before PV(k)
  so the PE never waits on the Act-engine exp.

All matmul inputs fp16; accumulation fp32 in PSUM.
"""
import numpy as np

import concourse.bass as bass
import concourse.tile as tile
import concourse.mybir as mybir
from concourse.bass_utils import run_bass_kernel_spmd
from concourse.tile_rust import add_dep_helper

N_CORES = 8
B, T, C = 4, 2048, 2048
H = 16                 # total heads
HPC = H // N_CORES     # heads per core = 2
D = C // H             # head dim = 128
HALF = D // 2
P = 128                # partitions
TG = 512               # attention query group
NTG = T // TG          # 4
NCC = C // P           # 16 contraction chunks
NSLICE = B * T // N_CORES  # 1024 output rows per core
TGP = 1024             # projection t-group
XCH = 512              # xt chunk columns
VW = D + 1             # 129: V plus fused ones column

FP = mybir.dt.float32
FP16 = mybir.dt.float16
EXP = mybir.ActivationFunctionType.Exp
SCALE = 1.0 / float(np.sqrt(D))

# ---------------------------------------------------------------------------
# Workaround: this container's walrus rejects >1 sync-wait per instruction.
# Hoist extras onto preceding same-engine NoOps (engine streams are in-order).
# ---------------------------------------------------------------------------
from concourse.vector_clock import ScopedClock


def _fixup_multiwaits(nc):
    moved = 0
    for fn in nc.m.functions:
        for bb in fn.blocks:
            insts = bb.instructions
            if not any(
                i.sync_info and i.sync_info.on_wait and len(i.sync_info.on_wait) > 1
                for i in insts
            ):
                continue
            new_insts = []
            for ins in insts:
                si = ins.sync_info
                if si is not None and si.on_wait and len(si.on_wait) > 1:
                    extra, keep = si.on_wait[:-1], si.on_wait[-1:]
                    for w in extra:
                        nop = mybir.InstNoOp(
                            name=nc.get_next_instruction_name(),
                            ins=[],
                            outs=[],
                            engine=ins.engine,
                        )
                        nop.sync_info = mybir.SyncInfo(on_wait=[w], on_update=[])
                        new_insts.append(nop)
                        moved += 1
                    si.on_wait = keep
                new_insts.append(ins)
            bb.instructions = new_insts
    return moved


def _patched_drain_and_barrier(self, tick_clock, wait_clock):
    nop = self.nc.sync.nop(nofuse=True)
    wait_clock.add_sem_waits(nop.ins, ScopedClock({None: tick_clock.global_clock}))
    w = nop.ins.sync_info.on_wait if nop.ins.sync_info else []
    while w and len(w) > 1:
        cond = w.pop()
        n2 = self.nc.sync.nop(nofuse=True)
        if n2.ins.sync_info is None:
            n2.ins.sync_info = mybir.SyncInfo(on_wait=[], on_update=[])
        n2.ins.sync_info.on_wait.append(cond)
    self.nc.sync.drain()
    self.nc.all_engine_barrier()
    assert self.sems is not None
    popped = self.nc._tile_sem_poison_stack.pop()
    assert popped is self._sem_poison
    self.nc.clear_and_free_semaphores(list(self.sems.allocated().values()))
    self.nc.all_engine_barrier()


tile.TileContext._drain_and_barrier = _patched_drain_and_barrier

# SBUF cap: tile_utils caps at 192KB/partition; cayman has 208 usable.
try:
    import concourse.tile_utils as _tile_utils

    if getattr(_tile_utils, "max_sbuf_usage", None) is not None:
        _tile_utils.max_sbuf_usage = 204 * 1024
except Exception:
    pass


# ---------------------------------------------------------------------------
# Device program
# ---------------------------------------------------------------------------
def build_program(reps: int = 1, mode: str = "full"):
    nc = bass.Bass()

    xT = nc.dram_tensor("xT", [B, C, T], FP16, kind="ExternalInput")
    wqT = nc.dram_tensor("wqT", [C, HPC * D], FP16, kind="ExternalInput")
    wkT = nc.dram_tensor("wkT", [C, HPC * D], FP16, kind="ExternalInput")
    wvT = nc.dram_tensor("wvT", [C, HPC * D], FP16, kind="ExternalInput")
    woT = nc.dram_tensor("woT", [C, C], FP16, kind="ExternalInput")
    cosC = nc.dram_tensor("cosC", [P, T], FP16, kind="ExternalInput")
    sinS = nc.dram_tensor("sinS", [P, T], FP16, kind="ExternalInput")
    maskd = nc.dram_tensor("maskd", [P, P], FP16, kind="ExternalInput")

    out_rows = nc.dram_tensor("out_rows", [NSLICE, C], FP, kind="ExternalOutput")

    with tile.TileContext(nc) as tc:
        with tc.tile_pool(name="const", bufs=1) as const:
            mask_s = const.tile([P, P], FP16, name="mask_s")
            wq_all = const.tile([P, NCC * HPC * D], FP16, name="wq_all")
            wk_all = const.tile([P, NCC * HPC * D], FP16, name="wk_all")
            wv_all = const.tile([P, NCC * HPC * D], FP16, name="wv_all")
            cos_s = const.tile([P, T], FP16, name="cos_s")
            sin_s = const.tile([P, T], FP16, name="sin_s")
            consts = dict(
                mask_s=mask_s, wq_all=wq_all, wk_all=wk_all, wv_all=wv_all,
                cos_s=cos_s, sin_s=sin_s,
                wqT=wqT, wkT=wkT, wvT=wvT, cosC=cosC, sinS=sinS, maskd=maskd,
            )
            for rep in range(reps):
                _emit_body(nc, tc, rep, xT, woT, out_rows, consts, mode=mode)

    moved = _fixup_multiwaits(nc)
    return nc, moved


def _load_weight(nc, dst, src):
    # src [C, W] DRAM row-major -> dst [P, NCC, W] (partition p = row%128)
    nc.sync.dma_start(
        dst[:].rearrange("p (cc w) -> p cc w", cc=NCC),
        src[:, :].rearrange("(cc p) w -> p cc w", p=P),
    )


def _emit_body(nc, tc, rep, xT, woT, out_rows, cst, mode="full"):
    ya_in = [
        nc.dram_tensor(f"ya_in_{rep}_{h}", [B * T, D], FP16) for h in range(HPC)
    ]
    ya_out = [
        nc.dram_tensor(f"ya_out_{rep}_{h}", [B * T, D], FP16) for h in range(HPC)
    ]

    ab = tc.alloc_tile_pool(name=f"ab{rep}", bufs=1)
    ps = tc.alloc_tile_pool(name=f"ps{rep}", bufs=1, space="PSUM")
    pa = tc.alloc_tile_pool(name=f"pa{rep}", bufs=1)

    w_all = {"q": cst["wq_all"], "k": cst["wk_all"]}
    qk = {}
    vall = {}

    def load_xt(b, tg, interleave=None):
        """Four chunk tiles of 4 cc each covering the tg's 1024 cols.
        interleave: optional list of thunks emitted between the DMAs
        (used at startup to slot weight loads into the queue)."""
        out = []
        for grp in range(4):
            t = pa.tile(
                [P, 4 * TGP], FP16, tag=f"xt{grp}", bufs=2,
                name=f"xt{grp}_{rep}_{b}_{tg}",
            )
            nc.sync.dma_start(
                t[:].rearrange("p (cc w) -> p cc w", cc=4),
                xT[b][
                    P * 4 * grp : P * 4 * (grp + 1), TGP * tg : TGP * (tg + 1)
                ].rearrange("(cc p) w -> p cc w", p=P),
            )
            if interleave and grp < len(interleave):
                interleave[grp]()
            out.append(t)
        return out

    def xsl(xts, cc, c0, w):
        """Moving slice of x chunk cc covering cols [c0, c0+w) of the tg group."""
        t = xts[cc // 4]
        cc0 = cc % 4
        return t[:, TGP * cc0 + c0 : TGP * cc0 + c0 + w]

    def proj(b, xts_by_tg):
        va = ab.tile([P, NCC * 2 * VW], FP16, tag="vall", bufs=4, name=f"va_{rep}_{b}")
        vall[b] = va
        nc.gpsimd.memset(
            va[:].rearrange("p (t x) -> p t x", x=VW)[:, :, D : D + 1], 1.0
        )
        for tg in range(T // TGP):
            xts = xts_by_tg[tg]
            for pj, h in (("q", 0), ("k", 0), ("q", 1), ("k", 1)):
                if tg == 0:
                    pool = pa if h == 0 else ab
                    qk[(pj, h, b)] = pool.tile(
                        [P, T], FP16, tag=f"{pj}T{h}", bufs=(2 if h == 0 else 4),
                        name=f"{pj}T{h}_{rep}_{b}",
                    )
                pmm = ps.tile(
                    [P, TGP], FP, tag="big2", bufs=2, name=f"pmm_{rep}_{b}_{tg}_{pj}{h}"
                )
                for half in range(2):
                    for cc in range(NCC):
                        nc.tensor.matmul(
                            pmm[:, XCH * half : XCH * (half + 1)],
                            w_all[pj][:, (HPC * D) * cc + D * h : (HPC * D) * cc + D * (h + 1)],
                            xsl(xts, cc, XCH * half, XCH),
                            start=(cc == 0),
                            stop=(cc == NCC - 1),
                        )
                # RoPE eviction: dst = pmm*C + rot(pmm)*S
                tc_sl = cst["cos_s"][:, TGP * tg : TGP * (tg + 1)]
                ts_sl = cst["sin_s"][:, TGP * tg : TGP * (tg + 1)]
                t1 = pa.tile([P, TGP], FP, tag="t1", bufs=2, name=f"t1_{rep}_{b}_{tg}_{pj}{h}")
                t2 = pa.tile([P, TGP], FP, tag="t2", bufs=2, name=f"t2_{rep}_{b}_{tg}_{pj}{h}")
                nc.vector.tensor_mul(t1[:], pmm[:], tc_sl)
                nc.vector.tensor_mul(t2[0:HALF, :], pmm[HALF:P, :], ts_sl[0:HALF, :])
                nc.vector.tensor_mul(t2[HALF:P, :], pmm[0:HALF, :], ts_sl[HALF:P, :])
                nc.vector.tensor_add(
                    qk[(pj, h, b)][:, TGP * tg : TGP * (tg + 1)], t1[:], t2[:]
                )
            # V projection: natural [t, d] with fused ones columns
            for vg in range(2):
                pv = ps.tile([P, TGP], FP, tag="big2", bufs=2, name=f"pv_{rep}_{b}_{tg}_{vg}")
                for ts4 in range(4):
                    for cc in range(NCC):
                        nc.tensor.matmul(
                            pv[:, (HPC * D) * ts4 : (HPC * D) * (ts4 + 1)],
                            xsl(xts, cc, XCH * vg + P * ts4, P),
                            cst["wv_all"][:, (HPC * D) * cc : (HPC * D) * (cc + 1)],
                            start=(cc == 0),
                            stop=(cc == NCC - 1),
                        )
                tch0 = 8 * tg + 4 * vg
                nc.scalar.copy(
                    va[:, 2 * VW * tch0 : 2 * VW * (tch0 + 4)].rearrange(
                        "p (t h d) -> p t h d", h=HPC, d=VW
                    )[:, :, :, 0:D],
                    pv[:].rearrange("p (t h d) -> p t h d", h=HPC, d=D),
                )

    last_staging = [None]

    def attn(b, h):
        qT = qk[("q", h, b)]
        kT = qk[("k", h, b)]
        va = vall[b]
        seq = [(g, pi) for g in range(NTG) for pi in range(2 * (g + 1))]
        po = {}

        def emit_pv(g, pi, pt):
            poA, poB = po[g]
            for half in range(2):
                i = 2 * pi + half
                jj = i - 4 * g
                for s in range(4):
                    if jj >= 0 and s < jj:
                        continue
                    pair, slot = divmod(s, 2)
                    dst = poA if pair == 0 else poB
                    # start=True clears the whole bank's has_written bits, so
                    # only the FIRST matmul into each pair-tile may set it —
                    # slot 1's first write relies on has_written=0 to store.
                    nc.tensor.matmul(
                        dst[:, VW * slot : VW * (slot + 1)],
                        pt[:, TG * half + P * s : TG * half + P * (s + 1)],
                        va[:, 2 * VW * i + VW * h : 2 * VW * i + VW * (h + 1)],
                        start=(i == 0 and slot == 0),
                        stop=(i == 4 * g + s),
                    )
            if pi == 2 * (g + 1) - 1:
                # group done: normalize + stage
                for pair in range(2):
                    pp = po[g][pair]
                    rc = ab.tile([P, 2], FP, tag="rc", bufs=2, name=f"rc_{rep}_{b}_{h}_{g}_{pair}")
                    ytn = ab.tile(
                        [P, 2 * D], FP16, tag=f"ytn{pair}", bufs=2,
                        name=f"ytn_{rep}_{b}_{h}_{g}_{pair}",
                    )
                    for slot in range(2):
                        nc.vector.reciprocal(
                            rc[:, slot : slot + 1], pp[:, VW * slot + D : VW * slot + D + 1]
                        )
                        nc.vector.tensor_scalar_mul(
                            ytn[:, D * slot : D * (slot + 1)],
                            pp[:, VW * slot : VW * slot + D],
                            rc[:, slot : slot + 1],
                        )
                    n0 = T * b + TG * g + 2 * P * pair
                    last_staging[0] = nc.sync.dma_start(
                        ya_in[h][n0 : n0 + 2 * P, :].rearrange("(s p) d -> p s d", s=2),
                        ytn[:].rearrange("p (s d) -> p s d", s=2),
                    )

        pending = None
        for g, pi in seq:
            if pi == 0:
                po[g] = (
                    ps.tile([P, 2 * VW], FP, tag="poA", bufs=2, name=f"poA_{rep}_{b}_{h}_{g}"),
                    ps.tile([P, 2 * VW], FP, tag="poB", bufs=2, name=f"poB_{rep}_{b}_{h}_{g}"),
                )
            pss = ps.tile([P, 2 * TG], FP, tag="big2", bufs=2, name=f"pss_{rep}_{b}_{h}_{g}_{pi}")
            for half in range(2):
                i = 2 * pi + half
                jj = i - 4 * g
                r = P * jj if jj >= 0 else 0
                nc.tensor.matmul(
                    pss[:, TG * half + r : TG * (half + 1)],
                    kT[:, P * i : P * (i + 1)],
                    qT[:, TG * g + r : TG * (g + 1)],
                    start=True,
                    stop=True,
                )
            pt = ab.tile([P, 2 * TG], FP16, tag="pt", bufs=2, name=f"pt_{rep}_{b}_{h}_{g}_{pi}")
            nc.scalar.activation(pt[:], pss[:], EXP, scale=SCALE)
            for half in range(2):
                i = 2 * pi + half
                jj = i - 4 * g
                if jj >= 0:
                    sl_ = slice(TG * half + P * jj, TG * half + P * (jj + 1))
                    nc.vector.tensor_mul(pt[:, sl_], pt[:, sl_], cst["mask_s"][:])
            if pending is not None:
                emit_pv(*pending)
            pending = (g, pi, pt)
        emit_pv(*pending)

    # ---- pass A: projections + h0 attention ---------------------------
    first = rep == 0
    inter = None
    if first:
        nc.sync.dma_start(cst["mask_s"][:], cst["maskd"][:])
        _load_weight(nc, cst["wq_all"], cst["wqT"])
        inter = [
            lambda: _load_weight(nc, cst["wk_all"], cst["wkT"]),
            lambda: (
                nc.sync.dma_start(cst["cos_s"][:], cst["cosC"][:]),
                nc.sync.dma_start(cst["sin_s"][:], cst["sinS"][:]),
            ),
            lambda: _load_weight(nc, cst["wv_all"], cst["wvT"]),
        ]
    xts0 = load_xt(0, 0, interleave=inter)
    xts1 = load_xt(0, 1)
    nxt = {0: xts0, 1: xts1}
    for b in range(B):
        cur = nxt
        proj(b, cur)
        if b + 1 < B:
            nxt = {tg: load_xt(b + 1, tg) for tg in range(2)}
        attn(b, 0)

    if mode == "full":
        nc.gpsimd.collective_compute(
            "AllToAll",
            mybir.AluOpType.bypass,
            replica_groups=[list(range(N_CORES))],
            ins=[ya_in[0][:]],
            outs=[ya_out[0][:]],
        )

    pa.release()
    wop = tc.alloc_tile_pool(name=f"wo{rep}", bufs=1, side="right")
    ytp = tc.alloc_tile_pool(name=f"ytp{rep}", bufs=1, side="right")
    op3 = tc.alloc_tile_pool(name=f"op3{rep}", bufs=1, side="right")

    # wo load rides the Activation HWDGE queue (SP would head-of-line-block
    # pass-B staging writes behind an 8MB transfer) and is held back until
    # pass-A staging is out, so it doesn't steal startup DMA bandwidth.
    wo_all = wop.tile([P, NCC * C], FP16, tag="wo", name=f"wo_{rep}")
    wo_dma = nc.scalar.dma_start(
        wo_all[:].rearrange("p (cc w) -> p cc w", cc=NCC),
        woT[:, :].rearrange("(cc p) w -> p cc w", p=P),
    )
    add_dep_helper(wo_dma.ins, last_staging[0].ins, reason="defer wo load")
    yt_all = {
        h: ytp.tile([P, N_CORES * NSLICE], FP16, tag=f"yt{h}", name=f"yt{h}_{rep}")
        for h in range(HPC)
    }

    def load_yt(h, after=None):
        for j in range(N_CORES):
            t = nc.sync.dma_start(
                yt_all[h][:, NSLICE * j : NSLICE * (j + 1)],
                ya_out[h][NSLICE * j : NSLICE * (j + 1), :],
                transpose=True,
            )
            if after is not None:
                # keep the greedy list scheduler from hoisting these into the
                # middle of pass B, where they head-of-line-block the SP DMA
                # queue (staging writes) behind the still-running AllToAll
                add_dep_helper(t.ins, after.ins, reason="defer yt load")

    # ---- pass B: h1 attention (AllToAll(h0) in flight) ----------------
    for b in range(B):
        attn(b, 1)
    # yt(h0) loads go after ALL pass-B staging writes: A2A(h0) is done by
    # now, so these fire immediately without blocking the SP queue.
    load_yt(0, after=last_staging[0])

    if mode == "full":
        nc.gpsimd.collective_compute(
            "AllToAll",
            mybir.AluOpType.bypass,
            replica_groups=[list(range(N_CORES))],
            ins=[ya_in[1][:]],
            outs=[ya_out[1][:]],
        )
    load_yt(1)

    ps.release()
    ps3 = tc.alloc_tile_pool(name=f"ps3{rep}", bufs=1, space="PSUM")

    # ---- output projection, h0-staggered ------------------------------
    jobs = [(jg, nt) for jg in range(C // TG) for nt in range(NSLICE // P)]
    pouts = {}

    def h_mms(idx, hs):
        jg, nt = jobs[idx]
        for j in range(N_CORES):
            ccg = HPC * j + hs
            nc.tensor.matmul(
                pouts[idx][:],
                yt_all[hs][:, NSLICE * j + P * nt : NSLICE * j + P * (nt + 1)],
                wo_all[:, C * ccg + TG * jg : C * ccg + TG * (jg + 1)],
                start=(hs == 0 and j == 0),
                stop=(hs == 1 and j == N_CORES - 1),
            )

    STAG = 8
    for idx in range(len(jobs) + STAG):
        if idx < len(jobs):
            pouts[idx] = ps3.tile([P, TG], FP, tag="pout", bufs=STAG, name=f"pout_{rep}_{idx}")
            h_mms(idx, 0)
        if idx >= STAG:
            k = idx - STAG
            h_mms(k, 1)
            jg, nt = jobs[k]
            ot = op3.tile([P, TG], FP, tag="ot", bufs=2, name=f"ot_{rep}_{k}")
            nc.scalar.copy(ot[:], pouts[k][:])
            nc.sync.dma_start(
                out_rows[P * nt : P * (nt + 1), TG * jg : TG * (jg + 1)], ot[:]
            )
            del pouts[k]

    ps3.release()
    op3.release()
    ytp.release()
    wop.release()
    ab.release()


# ---------------------------------------------------------------------------
# Host-side prep + execution
# ---------------------------------------------------------------------------
def _host_inputs(x, wq, wk, wv, wo):
    xT = np.ascontiguousarray(x.transpose(0, 2, 1)).astype(np.float16)
    woT = np.ascontiguousarray(wo.T).astype(np.float16)

    freqs = 1.0 / (10000.0 ** (np.arange(HALF, dtype=np.float32) / HALF))
    t = np.arange(T, dtype=np.float32)
    ang = freqs[:, None] * t[None, :]  # [64, T]
    cosC = np.concatenate([np.cos(ang), np.cos(ang)], axis=0).astype(np.float16)
    sinS = np.concatenate([-np.sin(ang), np.sin(ang)], axis=0).astype(np.float16)

    # maskd[k, q] = 1.0 iff q >= k
    maskd = np.triu(np.ones((P, P), dtype=np.float16))

    common = dict(xT=xT, woT=woT, cosC=cosC, sinS=sinS, maskd=maskd)
    in_maps = []
    for r in range(N_CORES):
        rows = slice(HPC * D * r, HPC * D * (r + 1))
        in_maps.append(
            dict(
                common,
                wqT=np.ascontiguousarray(wq[rows, :].T).astype(np.float16),
                wkT=np.ascontiguousarray(wk[rows, :].T).astype(np.float16),
                wvT=np.ascontiguousarray(wv[rows, :].T).astype(np.float16),
            )
        )
    return in_maps


_CACHED = {}


def _get_program(reps=1):
    if reps not in _CACHED:
        _CACHED[reps] = build_program(reps)[0]
    return _CACHED[reps]


def kernel(x, wq, wk, wv, wo):
    nc = _get_program(1)
    in_maps = _host_inputs(
        np.asarray(x, dtype=np.float32),
        np.asarray(wq, dtype=np.float32),
        np.asarray(wk, dtype=np.float32),
        np.asarray(wv, dtype=np.float32),
        np.asarray(wo, dtype=np.float32),
    )
    res = run_bass_kernel_spmd(nc, in_maps, list(range(N_CORES)))
    out = np.concatenate([res.results[r]["out_rows"] for r in range(N_CORES)], axis=0)
    return out.reshape(B, T, C)


# revision 14
# speedup vs baseline: 1.6269x; 1.1086x over previous
"""Tensor-parallel causal attention block for Trainium2 (8 NeuronCores).

Sharding: tensor-parallel across heads (2 heads/core) for QKV+attention,
then one AllToAll per local head (fp16, 2MB) to switch to row-parallel
for the output projection.

Key structure (v2):
- Head-major attention passes: all batches' h=0 attention runs first, so
  AllToAll(h0) overlaps the entire h=1 pass; phase 3 leads with h0
  accumulation (8-pout stagger) to cover AllToAll(h1).
- PV matmul uses exp(S^T) sub-blocks as the STATIONARY operand and
  [V | ones] as the moving operand: the softmax denominator comes out of
  the same matmul pass (column 128), eliminating the separate
  ones-row denominator matmuls and the reciprocal-broadcast matmul.
  Output is naturally [q, d]; normalization is a per-partition
  tensor_scalar multiply; staging is written [n, d]-major and
  re-transposed after the collective with the DMA transpose XBAR.
- RoPE fused into the QKV PSUM eviction as 4 full-partition DVE ops
  using host-built [cos;cos] / [-sin;sin] tables (fp16, loaded once).
- Bulk single-instruction DMAs (weights, x tiles, wo, yt) to cut
  descriptor-generation cost; DMA emission ordered so the first matmul
  starts after ~1.5MB of transfers.
- Attention is software-pipelined: scores(k+1) is emitted before PV(k)
  so the PE never waits on the Act-engine exp.

All matmul inputs fp16; accumulation fp32 in PSUM.
"""
import numpy as np

import concourse.bass as bass
import concourse.tile as tile
import concourse.mybir as mybir
from concourse.bass_utils import run_bass_kernel_spmd
from concourse.tile_rust import add_dep_helper

N_CORES = 8
B, T, C = 4, 2048, 2048
H = 16                 # total heads
HPC = H // N_CORES     # heads per core = 2
D = C // H             # head dim = 128
HALF = D // 2
P = 128                # partitions
TG = 512               # attention query group
NTG = T // TG          # 4
NCC = C // P           # 16 contraction chunks
NSLICE = B * T // N_CORES  # 1024 output rows per core
TGP = 1024             # projection t-group
XCH = 512              # xt chunk columns
VW = D + 1             # 129: V plus fused ones column

FP = mybir.dt.float32
FP16 = mybir.dt.float16
EXP = mybir.ActivationFunctionType.Exp
SCALE = 1.0 / float(np.sqrt(D))

# ---------------------------------------------------------------------------
# Workaround: this container's walrus rejects >1 sync-wait per instruction.
# Hoist extras onto preceding same-engine NoOps (engine streams are in-order).
# ---------------------------------------------------------------------------
from concourse.vector_clock import ScopedClock


def _fixup_multiwaits(nc):
    moved = 0
    for fn in nc.m.functions:
        for bb in fn.blocks:
            insts = bb.instructions
            if not any(
                i.sync_info and i.sync_info.on_wait and len(i.sync_info.on_wait) > 1
                for i in insts
            ):
                continue
            new_insts = []
            for ins in insts:
                si = ins.sync_info
                if si is not None and si.on_wait and len(si.on_wait) > 1:
                    extra, keep = si.on_wait[:-1], si.on_wait[-1:]
                    for w in extra:
                        nop = mybir.InstNoOp(
                            name=nc.get_next_instruction_name(),
                            ins=[],
                            outs=[],
                            engine=ins.engine,
                        )
                        nop.sync_info = mybir.SyncInfo(on_wait=[w], on_update=[])
                        new_insts.append(nop)
                        moved += 1
                    si.on_wait = keep
                new_insts.append(ins)
            bb.instructions = new_insts
    return moved


def _patched_drain_and_barrier(self, tick_clock, wait_clock):
    nop = self.nc.sync.nop(nofuse=True)
    wait_clock.add_sem_waits(nop.ins, ScopedClock({None: tick_clock.global_clock}))
    w = nop.ins.sync_info.on_wait if nop.ins.sync_info else []
    while w and len(w) > 1:
        cond = w.pop()
        n2 = self.nc.sync.nop(nofuse=True)
        if n2.ins.sync_info is None:
            n2.ins.sync_info = mybir.SyncInfo(on_wait=[], on_update=[])
        n2.ins.sync_info.on_wait.append(cond)
    self.nc.sync.drain()
    self.nc.all_engine_barrier()
    assert self.sems is not None
    popped = self.nc._tile_sem_poison_stack.pop()
    assert popped is self._sem_poison
    self.nc.clear_and_free_semaphores(list(self.sems.allocated().values()))
    self.nc.all_engine_barrier()


tile.TileContext._drain_and_barrier = _patched_drain_and_barrier

# SBUF cap: tile_utils caps at 192KB/partition; cayman has 208 usable.
try:
    import concourse.tile_utils as _tile_utils

    if getattr(_tile_utils, "max_sbuf_usage", None) is not None:
        _tile_utils.max_sbuf_usage = 204 * 1024
except Exception:
    pass


# ---------------------------------------------------------------------------
# Device program
# ---------------------------------------------------------------------------
def build_program(reps: int = 1, mode: str = "full"):
    nc = bass.Bass()

    xT = nc.dram_tensor("xT", [B, C, T], FP16, kind="ExternalInput")
    wqT = nc.dram_tensor("wqT", [C, HPC * D], FP16, kind="ExternalInput")
    wkT = nc.dram_tensor("wkT", [C, HPC * D], FP16, kind="ExternalInput")
    wvT = nc.dram_tensor("wvT", [C, HPC * D], FP16, kind="ExternalInput")
    woT = nc.dram_tensor("woT", [C, C], FP16, kind="ExternalInput")
    cosC = nc.dram_tensor("cosC", [P, T], FP16, kind="ExternalInput")
    sinS = nc.dram_tensor("sinS", [P, T], FP16, kind="ExternalInput")
    maskd = nc.dram_tensor("maskd", [P, P], FP16, kind="ExternalInput")

    out_rows = nc.dram_tensor("out_rows", [NSLICE, C], FP, kind="ExternalOutput")

    with tile.TileContext(nc) as tc:
        with tc.tile_pool(name="const", bufs=1) as const:
            mask_s = const.tile([P, P], FP16, name="mask_s")
            wq_all = const.tile([P, NCC * HPC * D], FP16, name="wq_all")
            wk_all = const.tile([P, NCC * HPC * D], FP16, name="wk_all")
            wv_all = const.tile([P, NCC * HPC * D], FP16, name="wv_all")
            cos_s = const.tile([P, T], FP16, name="cos_s")
            sin_s = const.tile([P, T], FP16, name="sin_s")
            consts = dict(
                mask_s=mask_s, wq_all=wq_all, wk_all=wk_all, wv_all=wv_all,
                cos_s=cos_s, sin_s=sin_s,
                wqT=wqT, wkT=wkT, wvT=wvT, cosC=cosC, sinS=sinS, maskd=maskd,
            )
            for rep in range(reps):
                _emit_body(nc, tc, rep, xT, woT, out_rows, consts, mode=mode)

    moved = _fixup_multiwaits(nc)
    return nc, moved


def _load_weight(nc, dst, src):
    # src [C, W] DRAM row-major -> dst [P, NCC, W] (partition p = row%128)
    nc.sync.dma_start(
        dst[:].rearrange("p (cc w) -> p cc w", cc=NCC),
        src[:, :].rearrange("(cc p) w -> p cc w", p=P),
    )


def _emit_body(nc, tc, rep, xT, woT, out_rows, cst, mode="full"):
    ya_in = [
        nc.dram_tensor(f"ya_in_{rep}_{h}", [B * T, D], FP16) for h in range(HPC)
    ]
    ya_out = [
        nc.dram_tensor(f"ya_out_{rep}_{h}", [B * T, D], FP16) for h in range(HPC)
    ]

    ab = tc.alloc_tile_pool(name=f"ab{rep}", bufs=1)
    ps = tc.alloc_tile_pool(name=f"ps{rep}", bufs=1, space="PSUM")
    pa = tc.alloc_tile_pool(name=f"pa{rep}", bufs=1)

    w_all = {"q": cst["wq_all"], "k": cst["wk_all"]}
    qk = {}
    vall = {}

    def load_xt(b, tg, interleave=None):
        """Four chunk tiles of 4 cc each covering the tg's 1024 cols.
        interleave: optional list of thunks emitted between the DMAs
        (used at startup to slot weight loads into the queue)."""
        out = []
        for grp in range(4):
            t = pa.tile(
                [P, 4 * TGP], FP16, tag=f"xt{grp}", bufs=2,
                name=f"xt{grp}_{rep}_{b}_{tg}",
            )
            nc.sync.dma_start(
                t[:].rearrange("p (cc w) -> p cc w", cc=4),
                xT[b][
                    P * 4 * grp : P * 4 * (grp + 1), TGP * tg : TGP * (tg + 1)
                ].rearrange("(cc p) w -> p cc w", p=P),
            )
            if interleave and grp < len(interleave):
                interleave[grp]()
            out.append(t)
        return out

    def xsl(xts, cc, c0, w):
        """Moving slice of x chunk cc covering cols [c0, c0+w) of the tg group."""
        t = xts[cc // 4]
        cc0 = cc % 4
        return t[:, TGP * cc0 + c0 : TGP * cc0 + c0 + w]

    def proj(b, xts_by_tg):
        va = ab.tile([P, NCC * 2 * VW], FP16, tag="vall", bufs=4, name=f"va_{rep}_{b}")
        vall[b] = va
        nc.gpsimd.memset(
            va[:].rearrange("p (t x) -> p t x", x=VW)[:, :, D : D + 1], 1.0
        )
        for tg in range(T // TGP):
            xts = xts_by_tg[tg]
            for pj, h in (("q", 0), ("k", 0), ("q", 1), ("k", 1)):
                if tg == 0:
                    pool = pa if h == 0 else ab
                    qk[(pj, h, b)] = pool.tile(
                        [P, T], FP16, tag=f"{pj}T{h}", bufs=(2 if h == 0 else 4),
                        name=f"{pj}T{h}_{rep}_{b}",
                    )
                pmm = ps.tile(
                    [P, TGP], FP, tag="big2", bufs=2, name=f"pmm_{rep}_{b}_{tg}_{pj}{h}"
                )
                for half in range(2):
                    for cc in range(NCC):
                        nc.tensor.matmul(
                            pmm[:, XCH * half : XCH * (half + 1)],
                            w_all[pj][:, (HPC * D) * cc + D * h : (HPC * D) * cc + D * (h + 1)],
                            xsl(xts, cc, XCH * half, XCH),
                            start=(cc == 0),
                            stop=(cc == NCC - 1),
                        )
                # RoPE eviction: dst = pmm*C + rot(pmm)*S
                tc_sl = cst["cos_s"][:, TGP * tg : TGP * (tg + 1)]
                ts_sl = cst["sin_s"][:, TGP * tg : TGP * (tg + 1)]
                t1 = pa.tile([P, TGP], FP, tag="t1", bufs=2, name=f"t1_{rep}_{b}_{tg}_{pj}{h}")
                t2 = pa.tile([P, TGP], FP, tag="t2", bufs=2, name=f"t2_{rep}_{b}_{tg}_{pj}{h}")
                nc.vector.tensor_mul(t1[:], pmm[:], tc_sl)
                nc.vector.tensor_mul(t2[0:HALF, :], pmm[HALF:P, :], ts_sl[0:HALF, :])
                nc.vector.tensor_mul(t2[HALF:P, :], pmm[0:HALF, :], ts_sl[HALF:P, :])
                nc.vector.tensor_add(
                    qk[(pj, h, b)][:, TGP * tg : TGP * (tg + 1)], t1[:], t2[:]
                )
            # V projection: natural [t, d] with fused ones columns
            for vg in range(2):
                pv = ps.tile([P, TGP], FP, tag="big2", bufs=2, name=f"pv_{rep}_{b}_{tg}_{vg}")
                for ts4 in range(4):
                    for cc in range(NCC):
                        nc.tensor.matmul(
                            pv[:, (HPC * D) * ts4 : (HPC * D) * (ts4 + 1)],
                            xsl(xts, cc, XCH * vg + P * ts4, P),
                            cst["wv_all"][:, (HPC * D) * cc : (HPC * D) * (cc + 1)],
                            start=(cc == 0),
                            stop=(cc == NCC - 1),
                        )
                tch0 = 8 * tg + 4 * vg
                nc.scalar.copy(
                    va[:, 2 * VW * tch0 : 2 * VW * (tch0 + 4)].rearrange(
                        "p (t h d) -> p t h d", h=HPC, d=VW
                    )[:, :, :, 0:D],
                    pv[:].rearrange("p (t h d) -> p t h d", h=HPC, d=D),
                )

    last_staging = [None]

    def attn(b, h):
        qT = qk[("q", h, b)]
        kT = qk[("k", h, b)]
        va = vall[b]
        seq = [(g, pi) for g in range(NTG) for pi in range(2 * (g + 1))]
        po = {}

        def emit_pv(g, pi, pt):
            poA, poB = po[g]
            for half in range(2):
                i = 2 * pi + half
                jj = i - 4 * g
                for s in range(4):
                    if jj >= 0 and s < jj:
                        continue
                    pair, slot = divmod(s, 2)
                    dst = poA if pair == 0 else poB
                    # start=True clears the whole bank's has_written bits, so
                    # only the FIRST matmul into each pair-tile may set it —
                    # slot 1's first write relies on has_written=0 to store.
                    nc.tensor.matmul(
                        dst[:, VW * slot : VW * (slot + 1)],
                        pt[:, TG * half + P * s : TG * half + P * (s + 1)],
                        va[:, 2 * VW * i + VW * h : 2 * VW * i + VW * (h + 1)],
                        start=(i == 0 and slot == 0),
                        stop=(i == 4 * g + s),
                    )
            if pi == 2 * (g + 1) - 1:
                # group done: normalize + stage
                for pair in range(2):
                    pp = po[g][pair]
                    rc = ab.tile([P, 2], FP, tag="rc", bufs=4, name=f"rc_{rep}_{b}_{h}_{g}_{pair}")
                    ytn = ab.tile(
                        [P, 2 * D], FP16, tag=f"ytn{pair}", bufs=4,
                        name=f"ytn_{rep}_{b}_{h}_{g}_{pair}",
                    )
                    for slot in range(2):
                        nc.vector.reciprocal(
                            rc[:, slot : slot + 1], pp[:, VW * slot + D : VW * slot + D + 1]
                        )
                        nc.vector.tensor_scalar_mul(
                            ytn[:, D * slot : D * (slot + 1)],
                            pp[:, VW * slot : VW * slot + D],
                            rc[:, slot : slot + 1],
                        )
                    n0 = T * b + TG * g + 2 * P * pair
                    last_staging[0] = nc.sync.dma_start(
                        ya_in[h][n0 : n0 + 2 * P, :].rearrange("(s p) d -> p s d", s=2),
                        ytn[:].rearrange("p (s d) -> p s d", s=2),
                    )

        pending = None
        for g, pi in seq:
            if pi == 0:
                po[g] = (
                    ps.tile([P, 2 * VW], FP, tag="poA", bufs=2, name=f"poA_{rep}_{b}_{h}_{g}"),
                    ps.tile([P, 2 * VW], FP, tag="poB", bufs=2, name=f"poB_{rep}_{b}_{h}_{g}"),
                )
            pss = ps.tile([P, 2 * TG], FP, tag="big2", bufs=2, name=f"pss_{rep}_{b}_{h}_{g}_{pi}")
            for half in range(2):
                i = 2 * pi + half
                jj = i - 4 * g
                r = P * jj if jj >= 0 else 0
                nc.tensor.matmul(
                    pss[:, TG * half + r : TG * (half + 1)],
                    kT[:, P * i : P * (i + 1)],
                    qT[:, TG * g + r : TG * (g + 1)],
                    start=True,
                    stop=True,
                )
            pt = ab.tile([P, 2 * TG], FP16, tag="pt", bufs=2, name=f"pt_{rep}_{b}_{h}_{g}_{pi}")
            nc.scalar.activation(pt[:], pss[:], EXP, scale=SCALE)
            for half in range(2):
                i = 2 * pi + half
                jj = i - 4 * g
                if jj >= 0:
                    sl_ = slice(TG * half + P * jj, TG * half + P * (jj + 1))
                    nc.vector.tensor_mul(pt[:, sl_], pt[:, sl_], cst["mask_s"][:])
            if pending is not None:
                emit_pv(*pending)
            pending = (g, pi, pt)
        emit_pv(*pending)

    # ---- pass A: projections + h0 attention ---------------------------
    first = rep == 0
    inter = None
    if first:
        nc.sync.dma_start(cst["mask_s"][:], cst["maskd"][:])
        _load_weight(nc, cst["wq_all"], cst["wqT"])
        inter = [
            lambda: _load_weight(nc, cst["wk_all"], cst["wkT"]),
            lambda: (
                nc.sync.dma_start(cst["cos_s"][:], cst["cosC"][:]),
                nc.sync.dma_start(cst["sin_s"][:], cst["sinS"][:]),
            ),
            lambda: _load_weight(nc, cst["wv_all"], cst["wvT"]),
        ]
    xts0 = load_xt(0, 0, interleave=inter)
    xts1 = load_xt(0, 1)
    nxt = {0: xts0, 1: xts1}
    for b in range(B):
        cur = nxt
        proj(b, cur)
        if b + 1 < B:
            nxt = {tg: load_xt(b + 1, tg) for tg in range(2)}
        attn(b, 0)

    if mode == "full":
        nc.gpsimd.collective_compute(
            "AllToAll",
            mybir.AluOpType.bypass,
            replica_groups=[list(range(N_CORES))],
            ins=[ya_in[0][:]],
            outs=[ya_out[0][:]],
        )

    pa.release()
    wop = tc.alloc_tile_pool(name=f"wo{rep}", bufs=1, side="right")
    ytp = tc.alloc_tile_pool(name=f"ytp{rep}", bufs=1, side="right")
    op3 = tc.alloc_tile_pool(name=f"op3{rep}", bufs=1, side="right")

    # wo load rides the Activation HWDGE queue (SP would head-of-line-block
    # pass-B staging writes behind an 8MB transfer) and is held back until
    # pass-A staging is out, so it doesn't steal startup DMA bandwidth.
    wo_all = wop.tile([P, NCC * C], FP16, tag="wo", name=f"wo_{rep}")
    for wg in range(4):
        wo_dma = nc.scalar.dma_start(
            wo_all[:, NCC * C // 4 * wg : NCC * C // 4 * (wg + 1)].rearrange(
                "p (cc w) -> p cc w", cc=NCC // 4
            ),
            woT[C // 4 * wg : C // 4 * (wg + 1), :].rearrange(
                "(cc p) w -> p cc w", p=P
            ),
        )
        add_dep_helper(wo_dma.ins, last_staging[0].ins, reason="defer wo load")
    yt_all = {
        h: ytp.tile([P, N_CORES * NSLICE], FP16, tag=f"yt{h}", name=f"yt{h}_{rep}")
        for h in range(HPC)
    }

    def load_yt(h, after=None):
        for j in range(N_CORES):
            t = nc.sync.dma_start(
                yt_all[h][:, NSLICE * j : NSLICE * (j + 1)],
                ya_out[h][NSLICE * j : NSLICE * (j + 1), :],
                transpose=True,
            )
            if after is not None:
                # keep the greedy list scheduler from hoisting these into the
                # middle of pass B, where they head-of-line-block the SP DMA
                # queue (staging writes) behind the still-running AllToAll
                add_dep_helper(t.ins, after.ins, reason="defer yt load")

    # ---- pass B: h1 attention (AllToAll(h0) in flight) ----------------
    for b in range(B):
        attn(b, 1)
    # yt(h0) loads go after ALL pass-B staging writes: A2A(h0) is done by
    # now, so these fire immediately without blocking the SP queue.
    load_yt(0, after=last_staging[0])

    if mode == "full":
        nc.gpsimd.collective_compute(
            "AllToAll",
            mybir.AluOpType.bypass,
            replica_groups=[list(range(N_CORES))],
            ins=[ya_in[1][:]],
            outs=[ya_out[1][:]],
        )
    load_yt(1)

    ps.release()
    ps3 = tc.alloc_tile_pool(name=f"ps3{rep}", bufs=1, space="PSUM")

    # ---- output projection, h0-staggered ------------------------------
    jobs = [(jg, nt) for jg in range(C // TG) for nt in range(NSLICE // P)]
    pouts = {}

    def h_mms(idx, hs):
        jg, nt = jobs[idx]
        for j in range(N_CORES):
            ccg = HPC * j + hs
            nc.tensor.matmul(
                pouts[idx][:],
                yt_all[hs][:, NSLICE * j + P * nt : NSLICE * j + P * (nt + 1)],
                wo_all[:, C * ccg + TG * jg : C * ccg + TG * (jg + 1)],
                start=(hs == 0 and j == 0),
                stop=(hs == 1 and j == N_CORES - 1),
            )

    STAG = 8
    for idx in range(len(jobs) + STAG):
        if idx < len(jobs):
            pouts[idx] = ps3.tile([P, TG], FP, tag="pout", bufs=STAG, name=f"pout_{rep}_{idx}")
            h_mms(idx, 0)
        if idx >= STAG:
            k = idx - STAG
            h_mms(k, 1)
            jg, nt = jobs[k]
            ot = op3.tile([P, TG], FP, tag="ot", bufs=2, name=f"ot_{rep}_{k}")
            nc.scalar.copy(ot[:], pouts[k][:])
            nc.sync.dma_start(
                out_rows[P * nt : P * (nt + 1), TG * jg : TG * (jg + 1)], ot[:]
            )
            del pouts[k]

    ps3.release()
    op3.release()
    ytp.release()
    wop.release()
    ab.release()


# ---------------------------------------------------------------------------
# Host-side prep + execution
# ---------------------------------------------------------------------------
def _host_inputs(x, wq, wk, wv, wo):
    xT = np.ascontiguousarray(x.transpose(0, 2, 1)).astype(np.float16)
    woT = np.ascontiguousarray(wo.T).astype(np.float16)

    freqs = 1.0 / (10000.0 ** (np.arange(HALF, dtype=np.float32) / HALF))
    t = np.arange(T, dtype=np.float32)
    ang = freqs[:, None] * t[None, :]  # [64, T]
    cosC = np.concatenate([np.cos(ang), np.cos(ang)], axis=0).astype(np.float16)
    sinS = np.concatenate([-np.sin(ang), np.sin(ang)], axis=0).astype(np.float16)

    # maskd[k, q] = 1.0 iff q >= k
    maskd = np.triu(np.ones((P, P), dtype=np.float16))

    common = dict(xT=xT, woT=woT, cosC=cosC, sinS=sinS, maskd=maskd)
    in_maps = []
    for r in range(N_CORES):
        rows = slice(HPC * D * r, HPC * D * (r + 1))
        in_maps.append(
            dict(
                common,
                wqT=np.ascontiguousarray(wq[rows, :].T).astype(np.float16),
                wkT=np.ascontiguousarray(wk[rows, :].T).astype(np.float16),
                wvT=np.ascontiguousarray(wv[rows, :].T).astype(np.float16),
            )
        )
    return in_maps


_CACHED = {}


def _get_program(reps=1):
    if reps not in _CACHED:
        _CACHED[reps] = build_program(reps)[0]
    return _CACHED[reps]


def kernel(x, wq, wk, wv, wo):
    nc = _get_program(1)
    in_maps = _host_inputs(
        np.asarray(x, dtype=np.float32),
        np.asarray(wq, dtype=np.float32),
        np.asarray(wk, dtype=np.float32),
        np.asarray(wv, dtype=np.float32),
        np.asarray(wo, dtype=np.float32),
    )
    res = run_bass_kernel_spmd(nc, in_maps, list(range(N_CORES)))
    out = np.concatenate([res.results[r]["out_rows"] for r in range(N_CORES)], axis=0)
    return out.reshape(B, T, C)
